# revision 18
# baseline (speedup 1.0000x reference)
"""BiLSTM-CRF Trainium2 kernel (8 NeuronCores, SPMD).

Model quirk (faithful to reference): the LSTM scans over the batch dim
B=32 as "time" with S=512 as its batch; the CRF/viterbi scans over S=512
with B=32 as batch.

Sharding: data-parallel over S for the LSTM (64 positions/core, 8 cores),
feats AllGather, then the (inherently serial) viterbi delta recursion runs
replicated on every core; the host backtracks the path from the device DP
tables (O(S*B) gather work).

Precision: all three matmuls (x-proj, h-recurrence, w_out) in fp16 --
validated against the reference inputs to give 0 viterbi path flips
(bf16 flips ~220 path entries; fp16 is safe). Everything else fp32.
"""

import sys

sys.path.insert(0, "/opt/trn_rl_repo")

import contextlib

import numpy as np

import concourse.bass as bass
import concourse.mybir as mybir
import concourse.tile as tile
from concourse import bacc
from concourse.bass_utils import run_bass_kernel_spmd
from concourse.masks import make_identity

F32 = mybir.dt.float32
F16 = mybir.dt.float16
I32 = mybir.dt.int32
AX = mybir.AxisListType
OP = mybir.AluOpType
ACTF = mybir.ActivationFunctionType

# model dims
V, E, HD, T = 100000, 256, 512, 11
H = HD // 2           # 256
B, S = 32, 512        # B = lstm time, S = viterbi time
G = 4 * H             # 1024 gates per dir
START, STOP = 9, 10
NEG = -10000.0

NCORES = 8
SB = S // NCORES      # 64 s-positions per core
R = B * SB            # 2048 (t, s) pairs per core
KC = 2                # hidden/embed 256 = 2 chunks of 128
GC = 8                # 1024 gates = 8 chunks of 128
NQ = R // 512         # 4 moving-dim chunks for the input projection
VCH = 64              # viterbi feats chunk

_CACHE = {}


def _build():
    nc = bacc.Bacc(None, target_bir_lowering=False, debug=False)

    # ---- per-core inputs ----
    sent_d = nc.declare_dram_parameter("sent", [R], I32, isOutput=False)
    h0_d = nc.declare_dram_parameter("h0t", [2, 128, KC, SB], F16, isOutput=False)
    c0_d = nc.declare_dram_parameter("c0t", [2, 128, KC, SB], F32, isOutput=False)
    # ---- replicated inputs ----
    embed_d = nc.declare_dram_parameter("embed", [V, E], F32, isOutput=False)
    wih_d = nc.declare_dram_parameter("wihT", [2, E, G], F16, isOutput=False)
    whh_d = nc.declare_dram_parameter("whhT", [2, H, G], F16, isOutput=False)
    biasg_d = nc.declare_dram_parameter("biasg", [2, 128, GC], F32, isOutput=False)
    wout_d = nc.declare_dram_parameter("woutT", [128, 4, T], F16, isOutput=False)
    bout_d = nc.declare_dram_parameter("bout", [T, 1], F32, isOutput=False)
    ttile_d = nc.declare_dram_parameter("transtile", [B, T, T], F32, isOutput=False)
    delta0_d = nc.declare_dram_parameter("delta0", [B, T], F32, isOutput=False)
    # ---- outputs ----
    deltas_d = nc.declare_dram_parameter("deltas", [B, S * T], F32, isOutput=True)

    with tile.TileContext(nc) as tc, contextlib.ExitStack() as ctx:
        const = ctx.enter_context(tc.tile_pool(name="const", bufs=1))
        big = ctx.enter_context(tc.tile_pool(name="big", bufs=1))
        dram = ctx.enter_context(tc.tile_pool(name="dram", bufs=1, space="DRAM"))

        identity = const.tile([128, 128], F32)
        make_identity(nc, identity[:])

        whh_sb = const.tile([128, 2, KC, G], F16)
        nc.sync.dma_start(whh_sb[:], whh_d[:].rearrange("d (kc p) g -> p d kc g", p=128))
        biasg_sb = const.tile([128, 2, GC], F32)
        nc.sync.dma_start(biasg_sb[:], biasg_d[:].rearrange("d p g -> p d g"))
        wout_sb = const.tile([128, 4, T], F16)
        nc.sync.dma_start(wout_sb[:], wout_d[:])
        bout_sb = const.tile([T, 1], F32)
        nc.sync.dma_start(bout_sb[:], bout_d[:])
        trans_sb = const.tile([B, T, T], F32)
        nc.sync.dma_start(trans_sb[:], ttile_d[:])

        hAll = big.tile([128, 2, KC, B + 2, SB], F16)
        cT = big.tile([128, 2, KC, SB], F32)
        dwin = big.tile([B, S * T], F32)

        # input projection lives in DRAM; streamed per-step in the recurrence
        P_dram = dram.tile([2, B, 128, GC, 64], F32)

        # ================= stage A-C: gather, transpose, input proj ==========
        with tc.tile_pool(name="early", bufs=1) as early, \
             tc.tile_pool(name="psA", bufs=1, space="PSUM") as psum:
            idx_sb = early.tile([128, 16], I32)
            nc.sync.dma_start(idx_sb[:], sent_d[:].rearrange("(k p) -> p k", p=128))

            xT = early.tile([128, KC, R], F16)  # [e-chunk partition, (ec, r)]
            for k in range(16):
                xtmp = early.tile([128, E], F32, tag="xtmp", bufs=3)
                nc.gpsimd.indirect_dma_start(
                    out=xtmp[:],
                    out_offset=None,
                    in_=embed_d[:, :],
                    in_offset=bass.IndirectOffsetOnAxis(ap=idx_sb[:, k : k + 1], axis=0),
                )
                for ec in range(KC):
                    pt = psum.tile([128, 128], F32, tag="tpose", bufs=2)
                    nc.tensor.transpose(pt[:], xtmp[:, ec * 128 : (ec + 1) * 128], identity[:])
                    if (k + ec) % 2 == 0:
                        nc.vector.tensor_copy(xT[:, ec, k * 128 : (k + 1) * 128], pt[:])
                    else:
                        nc.scalar.copy(xT[:, ec, k * 128 : (k + 1) * 128], pt[:])

            wih_sb = early.tile([128, 2, KC, G], F16)
            nc.sync.dma_start(wih_sb[:], wih_d[:].rearrange("d (kc p) g -> p d kc g", p=128))

            for d in range(2):
                for gc in range(GC):
                    for nq in range(NQ):
                        pp = psum.tile([128, 512], F32, tag="pp", bufs=2)
                        for kc in range(KC):
                            nc.tensor.matmul(
                                pp[:],
                                lhsT=wih_sb[:, d, kc, gc * 128 : (gc + 1) * 128],
                                rhs=xT[:, kc, nq * 512 : (nq + 1) * 512],
                                start=(kc == 0),
                                stop=(kc == KC - 1),
                            )
                        ptmp = early.tile([128, 8, 64], F32, tag="ptmp", bufs=3)
                        if (gc + nq) % 2 == 0:
                            nc.vector.tensor_scalar_add(
                                ptmp[:].rearrange("p t s -> p (t s)"), pp[:],
                                biasg_sb[:, d, gc : gc + 1])
                        else:
                            nc.scalar.activation(
                                ptmp[:].rearrange("p t s -> p (t s)"), pp[:], ACTF.Identity,
                                bias=biasg_sb[:, d, gc : gc + 1], scale=1.0,
                            )
                        # pp rows r = nq*512 + (t', s): t = 8*nq + t'
                        nc.sync.dma_start(
                            P_dram[d, 8 * nq : 8 * nq + 8, :, gc, :].rearrange(
                                "t p s -> p t s"),
                            ptmp[:],
                        )

        # ================= stage D: recurrence (fp16 matmuls) ================
        for d in range(2):
            init_slot = 0 if d == 0 else B + 1
            for kc in range(KC):
                nc.sync.dma_start(hAll[:, d, kc, init_slot, :], h0_d[d, :, kc, :])
                nc.sync.dma_start(cT[:, d, kc, :], c0_d[d, :, kc, :])

        with tc.tile_pool(name="rec", bufs=1) as rec, \
             tc.tile_pool(name="psD", bufs=1, space="PSUM") as psum:
            for step in range(B):
                for d in range(2):
                    tl = step if d == 0 else B - 1 - step  # x/P time index
                    rd_slot = step if d == 0 else B + 1 - step
                    wr_slot = step + 1 if d == 0 else B - step
                    pslice = rec.tile([128, GC, 64], F32, tag=f"pslice{d}", bufs=3)
                    nc.sync.dma_start(pslice[:], P_dram[d, tl, :, :, :])
                    gp = psum.tile([128, 512], F32, tag=f"gp{d}", bufs=2)
                    for gc in range(GC):
                        for kc in range(KC):
                            nc.tensor.matmul(
                                gp[:, gc * 64 : (gc + 1) * 64],
                                lhsT=whh_sb[:, d, kc, gc * 128 : (gc + 1) * 128],
                                rhs=hAll[:, d, kc, rd_slot, :],
                                start=(kc == 0),
                                stop=(kc == KC - 1),
                            )
                    pre = rec.tile([128, GC, 64], F32, tag=f"pre{d}", bufs=3)
                    nc.vector.tensor_tensor(
                        pre[:],
                        gp[:].rearrange("p (g s) -> p g s", g=GC),
                        pslice[:],
                        op=OP.add,
                    )
                    gates = rec.tile([128, 512], F32, tag=f"gates{d}", bufs=3)
                    prefl = pre[:].rearrange("p g s -> p (g s)")
                    nc.scalar.activation(gates[:, 0:384], prefl[:, 0:384], ACTF.Sigmoid)
                    nc.scalar.activation(gates[:, 384:512], prefl[:, 384:512], ACTF.Tanh)
                    # gate layout (after host row-permute): i f o g, each 128 wide
                    ct = cT[:, d, :, :].rearrange("p kc s -> p (kc s)")
                    tmp = rec.tile([128, 128], F32, tag=f"tmp{d}", bufs=3)
                    nc.vector.tensor_tensor(tmp[:], gates[:, 0:128], gates[:, 384:512], op=OP.mult)
                    nc.vector.tensor_tensor(ct, gates[:, 128:256], ct, op=OP.mult)
                    nc.vector.tensor_tensor(ct, ct, tmp[:], op=OP.add)
                    tc2 = rec.tile([128, 128], F32, tag=f"tc{d}", bufs=3)
                    nc.scalar.activation(tc2[:], ct, ACTF.Tanh)
                    nc.vector.tensor_tensor(
                        hAll[:, d, :, wr_slot, :],
                        gates[:, 256:384].rearrange("p (kc s) -> p kc s", kc=KC),
                        tc2[:].rearrange("p (kc s) -> p kc s", kc=KC),
                        op=OP.mult,
                    )

        # ================= stage E-F: feats, transpose, AllGather =============
        fb_local = dram.tile([R, T], F32)
        with tc.tile_pool(name="fstage", bufs=1) as fs, \
             tc.tile_pool(name="psE", bufs=1, space="PSUM") as psum:
            feats_sb = fs.tile([T, R], F32)
            for nq in range(NQ):
                fp = psum.tile([T, 512], F32, tag="fpsum", bufs=2)
                for d in range(2):
                    for kc in range(KC):
                        q = d * 2 + kc
                        hflat = hAll[:, d, kc, 1 : B + 1, :].rearrange("p t s -> p (t s)")
                        nc.tensor.matmul(
                            fp[:],
                            lhsT=wout_sb[:, q, :],
                            rhs=hflat[:, nq * 512 : (nq + 1) * 512],
                            start=(q == 0),
                            stop=(q == 3),
                        )
                nc.scalar.activation(
                    feats_sb[:, nq * 512 : (nq + 1) * 512], fp[:], ACTF.Identity,
                    bias=bout_sb[:], scale=1.0,
                )

            ftr = fs.tile([128, 16, T], F32)
            for k in range(16):
                pt2 = psum.tile([128, T], F32, tag="ftpose", bufs=2)
                nc.tensor.transpose(
                    pt2[:], feats_sb[:, k * 128 : (k + 1) * 128], identity[:T, :T]
                )
                if k % 2 == 0:
                    nc.vector.tensor_copy(ftr[:, k, :], pt2[:])
                else:
                    nc.scalar.copy(ftr[:, k, :], pt2[:])
            nc.sync.dma_start(fb_local[:].rearrange("(k p) j -> p k j", p=128), ftr[:])

        # M[b, s, j, i] = trans[j, i] + feat[b, 64c + s, j] for this core's block
        Mown_dram = dram.tile([B, SB * T * T], F32)
        with tc.tile_pool(name="mstage", bufs=1) as ms:
            fown = ms.tile([B, SB, T], F32)
            nc.sync.dma_start(
                fown[:],
                fb_local[:].rearrange("(b s) j -> b s j", b=B),
            )
            Mown = ms.tile([B, SB, T, T], F32)
            nc.vector.tensor_tensor(
                Mown[:],
                trans_sb[:, None, :, :].to_broadcast([B, SB, T, T]),
                fown[:, :, :, None].to_broadcast([B, SB, T, T]),
                op=OP.add,
            )
            nc.sync.dma_start(
                Mown_dram[:], Mown[:].rearrange("b s j i -> b (s j i)")
            )
        M_full = dram.tile([NCORES, B, SB * T * T], F32, addr_space="Shared")
        nc.gpsimd.collective_compute(
            "AllGather",
            OP.bypass,
            replica_groups=[list(range(NCORES))],
            ins=[Mown_dram[:].opt()],
            outs=[M_full[:].opt()],
        )

        # ================= stage G: viterbi delta scan (replicated) ==========
        # 3-op scan, all on DVE:
        #   sc[b,j,i] = trans[j,i] + delta_{s-1}[b,i];  red = max_i sc;
        #   delta_s   = red + feat_s
        nc.sync.dma_start(dwin[:, 0:T], delta0_d[:])
        with tc.tile_pool(name="vit", bufs=1) as vit:
            for ch in range(S // VCH):
                Mch = vit.tile([B, VCH, T, T], F32, tag="Mch", bufs=3)
                nc.sync.dma_start(
                    Mch[:].rearrange("b s j i -> b (s j i)"), M_full[ch, :, :]
                )
                for sl in range(VCH):
                    s = ch * VCH + sl
                    if s == 0:
                        continue
                    sc = vit.tile([B, T, T], F32, tag="sc", bufs=6)
                    nc.vector.tensor_tensor(
                        sc[:],
                        Mch[:, sl, :, :],
                        dwin[:, (s - 1) * T : s * T][:, None, :].to_broadcast([B, T, T]),
                        op=OP.add,
                    )
                    nc.vector.tensor_reduce(
                        dwin[:, s * T : (s + 1) * T], sc[:], axis=AX.X, op=OP.max
                    )

        nc.sync.dma_start(deltas_d[:], dwin[:])

    nc.finalize()
    return nc


def _pack_inputs(inputs):
    f32 = np.float32
    sentence = np.asarray(inputs["sentence"]).astype(np.int32)
    embed = np.ascontiguousarray(np.asarray(inputs["embed"], f32))
    h0 = np.asarray(inputs["h0"], f32)
    c0 = np.asarray(inputs["c0"], f32)
    w_out = np.asarray(inputs["w_out"], f32)
    b_out = np.asarray(inputs["b_out"], f32)
    transitions = np.asarray(inputs["transitions"], f32)

    # gate row permutation: torch (i, f, g, o) -> device (i, f, o, g)
    perm = np.concatenate([
        np.arange(0, 256), np.arange(256, 512),
        np.arange(768, 1024), np.arange(512, 768),
    ])

    def pack_dir(w_ih, w_hh, b_ih, b_hh):
        w_ih = np.asarray(w_ih, f32)[perm]
        w_hh = np.asarray(w_hh, f32)[perm]
        bias = (np.asarray(b_ih, f32) + np.asarray(b_hh, f32))[perm]
        return (
            np.ascontiguousarray(w_ih.T),                       # (E, G)
            np.ascontiguousarray(w_hh.T),                       # (H, G)
            np.ascontiguousarray(bias.reshape(GC, 128).T),      # (128, GC)
        )

    wihT_f, whhT_f, biasg_f = pack_dir(inputs["w_ih_f"], inputs["w_hh_f"],
                                       inputs["b_ih_f"], inputs["b_hh_f"])
    wihT_b, whhT_b, biasg_b = pack_dir(inputs["w_ih_b"], inputs["w_hh_b"],
                                       inputs["b_ih_b"], inputs["b_hh_b"])
    wihT = np.ascontiguousarray(np.stack([wihT_f, wihT_b])).astype(np.float16)
    whhT = np.ascontiguousarray(np.stack([whhT_f, whhT_b])).astype(np.float16)
    biasg = np.ascontiguousarray(np.stack([biasg_f, biasg_b]))

    # w_out (T, HD): lhsT chunks [128, q, T], q = d*2+kc over the hf|hb concat
    woutT = np.ascontiguousarray(
        w_out.T.reshape(4, 128, T).transpose(1, 0, 2)).astype(np.float16)
    boutc = np.ascontiguousarray(b_out.reshape(T, 1))
    transtile = np.ascontiguousarray(np.broadcast_to(transitions[None], (B, T, T)))
    delta0 = np.full((B, T), NEG, f32)
    delta0[:, START] = 0.0

    shared = dict(embed=embed, wihT=wihT, whhT=whhT, biasg=biasg,
                  woutT=woutT, bout=boutc, transtile=transtile, delta0=delta0)

    in_maps = []
    for c in range(NCORES):
        sl = slice(c * SB, (c + 1) * SB)
        sent = np.ascontiguousarray(sentence[:, sl].reshape(-1))  # (R,) t-major

        def pack_state(st):
            sh = np.asarray(st, f32)[:, sl, :]                      # (2, SB, H)
            sh = sh.transpose(0, 2, 1).reshape(2, KC, 128, SB)      # (2, kc, p, s)
            return np.ascontiguousarray(sh.transpose(0, 2, 1, 3))   # (2, p, kc, s)

        m = dict(shared)
        m["sent"] = sent
        m["h0t"] = pack_state(h0).astype(np.float16)
        m["c0t"] = pack_state(c0)
        in_maps.append(m)
    return in_maps, transitions


def _run(inputs, trace=False):
    if "nc" not in _CACHE:
        _CACHE["nc"] = _build()
    nc = _CACHE["nc"]
    in_maps, transitions = _pack_inputs(inputs)
    res = run_bass_kernel_spmd(nc, in_maps, core_ids=list(range(NCORES)), trace=trace)
    return res, transitions


def kernel(**inputs):
    res, transitions = _run(inputs, trace=False)
    deltas = res.results[0]["deltas"].reshape(B, S, T)
    return _finish(deltas, transitions)


def _finish(deltas, transitions):
    score = deltas[:, S - 1].max(-1).astype(np.float32)
    nxt = deltas[:, S - 1].argmax(-1)
    path = np.empty((B, S), np.int64)
    path[:, S - 1] = nxt
    tr = np.asarray(transitions, np.float32)
    for s in range(S - 1, 0, -1):
        scb = tr[nxt] + deltas[:, s - 1]
        nxt = scb.argmax(-1)
        path[:, s - 1] = nxt
    return score, path.astype(np.int32)


# revision 20
# speedup vs baseline: 1.0537x; 1.0537x over previous
"""BiLSTM-CRF Trainium2 kernel (8 NeuronCores, SPMD).

Model quirk (faithful to reference): the LSTM scans over the batch dim
B=32 as "time" with S=512 as its batch; the CRF/viterbi scans over S=512
with B=32 as batch.

Sharding: data-parallel over S for the LSTM (64 positions/core, 8 cores),
feats AllGather, then the (inherently serial) viterbi delta recursion runs
replicated on every core; the host backtracks the path from the device DP
tables (O(S*B) gather work).

Precision: all three matmuls (x-proj, h-recurrence, w_out) in fp16 --
validated against the reference inputs to give 0 viterbi path flips
(bf16 flips ~220 path entries; fp16 is safe). Everything else fp32.
"""

import sys

sys.path.insert(0, "/opt/trn_rl_repo")

import contextlib

import numpy as np

import concourse.bass as bass
import concourse.mybir as mybir
import concourse.tile as tile
from concourse import bacc
from concourse.bass_utils import run_bass_kernel_spmd
from concourse.masks import make_identity

F32 = mybir.dt.float32
F16 = mybir.dt.float16
I32 = mybir.dt.int32
AX = mybir.AxisListType
OP = mybir.AluOpType
ACTF = mybir.ActivationFunctionType

# model dims
V, E, HD, T = 100000, 256, 512, 11
H = HD // 2           # 256
B, S = 32, 512        # B = lstm time, S = viterbi time
G = 4 * H             # 1024 gates per dir
START, STOP = 9, 10
NEG = -10000.0

NCORES = 8
SB = S // NCORES      # 64 s-positions per core
R = B * SB            # 2048 (t, s) pairs per core
KC = 2                # hidden/embed 256 = 2 chunks of 128
GC = 8                # 1024 gates = 8 chunks of 128
NQ = R // 512         # 4 moving-dim chunks for the input projection
VCH = 64              # viterbi feats chunk

_CACHE = {}


def _build():
    nc = bacc.Bacc(None, target_bir_lowering=False, debug=False)

    # ---- per-core inputs ----
    sent_d = nc.declare_dram_parameter("sent", [R], I32, isOutput=False)
    h0_d = nc.declare_dram_parameter("h0t", [2, 128, KC, SB], F16, isOutput=False)
    c0_d = nc.declare_dram_parameter("c0t", [2, 128, KC, SB], F32, isOutput=False)
    # ---- replicated inputs ----
    embed_d = nc.declare_dram_parameter("embed", [V, E], F32, isOutput=False)
    wih_d = nc.declare_dram_parameter("wihT", [2, E, G], F16, isOutput=False)
    whh_d = nc.declare_dram_parameter("whhT", [2, H, G], F16, isOutput=False)
    biasg_d = nc.declare_dram_parameter("biasg", [2, 128, GC], F32, isOutput=False)
    wout_d = nc.declare_dram_parameter("woutT", [128, 4, T], F16, isOutput=False)
    bout_d = nc.declare_dram_parameter("bout", [T, 1], F32, isOutput=False)
    ttile_d = nc.declare_dram_parameter("transtile", [B, T, T], F32, isOutput=False)
    delta0_d = nc.declare_dram_parameter("delta0", [B, T], F32, isOutput=False)
    # ---- outputs ----
    deltas_d = nc.declare_dram_parameter("deltas", [B, S * T], F32, isOutput=True)

    with tile.TileContext(nc) as tc, contextlib.ExitStack() as ctx:
        const = ctx.enter_context(tc.tile_pool(name="const", bufs=1))
        big = ctx.enter_context(tc.tile_pool(name="big", bufs=1))
        dram = ctx.enter_context(tc.tile_pool(name="dram", bufs=1, space="DRAM"))

        identity = const.tile([128, 128], F32)
        make_identity(nc, identity[:])

        whh_sb = const.tile([128, 2, KC, G], F16)
        nc.sync.dma_start(whh_sb[:], whh_d[:].rearrange("d (kc p) g -> p d kc g", p=128))
        biasg_sb = const.tile([128, 2, GC], F32)
        nc.sync.dma_start(biasg_sb[:], biasg_d[:].rearrange("d p g -> p d g"))
        wout_sb = const.tile([128, 4, T], F16)
        nc.sync.dma_start(wout_sb[:], wout_d[:])
        bout_sb = const.tile([T, 1], F32)
        nc.sync.dma_start(bout_sb[:], bout_d[:])
        trans_sb = const.tile([B, T, T], F32)
        nc.sync.dma_start(trans_sb[:], ttile_d[:])

        hAll = big.tile([128, 2, KC, B + 2, SB], F16)
        cT = big.tile([128, 2, KC, SB], F32)
        dwin = big.tile([B, S * T], F32)

        # input projection lives in DRAM; streamed per-step in the recurrence
        P_dram = dram.tile([2, B, 128, GC, 64], F32)

        # ================= stage A-C: gather, transpose, input proj ==========
        with tc.tile_pool(name="early", bufs=1) as early, \
             tc.tile_pool(name="psA", bufs=1, space="PSUM") as psum:
            idx_sb = early.tile([128, 16], I32)
            nc.sync.dma_start(idx_sb[:], sent_d[:].rearrange("(k p) -> p k", p=128))

            xT = early.tile([128, KC, R], F16)  # [e-chunk partition, (ec, r)]
            for k in range(16):
                xtmp = early.tile([128, E], F32, tag="xtmp", bufs=3)
                nc.gpsimd.indirect_dma_start(
                    out=xtmp[:],
                    out_offset=None,
                    in_=embed_d[:, :],
                    in_offset=bass.IndirectOffsetOnAxis(ap=idx_sb[:, k : k + 1], axis=0),
                )
                for ec in range(KC):
                    pt = psum.tile([128, 128], F32, tag="tpose", bufs=2)
                    nc.tensor.transpose(pt[:], xtmp[:, ec * 128 : (ec + 1) * 128], identity[:])
                    if (k + ec) % 2 == 0:
                        nc.vector.tensor_copy(xT[:, ec, k * 128 : (k + 1) * 128], pt[:])
                    else:
                        nc.scalar.copy(xT[:, ec, k * 128 : (k + 1) * 128], pt[:])

            wih_sb = early.tile([128, 2, KC, G], F16)
            nc.sync.dma_start(wih_sb[:], wih_d[:].rearrange("d (kc p) g -> p d kc g", p=128))

            # keep the PE HAM-warm through the gather phase: idle gaps >3.4us
            # re-throttle the clock to 1.2 GHz and stage C would start cold
            warm = early.tile([128, 512], F16, tag="warm")
            nc.gpsimd.memset(warm[:], 0)
            wpp = psum.tile([128, 512], F32, tag="wpp", bufs=1)
            for w in range(24):
                nc.tensor.matmul(wpp[:], lhsT=warm[:, 0:128], rhs=warm[:],
                                 start=True, stop=True)

            for d in range(2):
                for gc in range(GC):
                    for nq in range(NQ):
                        pp = psum.tile([128, 512], F32, tag="pp", bufs=2)
                        for kc in range(KC):
                            nc.tensor.matmul(
                                pp[:],
                                lhsT=wih_sb[:, d, kc, gc * 128 : (gc + 1) * 128],
                                rhs=xT[:, kc, nq * 512 : (nq + 1) * 512],
                                start=(kc == 0),
                                stop=(kc == KC - 1),
                            )
                        ptmp = early.tile([128, 8, 64], F32, tag="ptmp", bufs=3)
                        if (gc + nq) % 2 == 0:
                            nc.vector.tensor_scalar_add(
                                ptmp[:].rearrange("p t s -> p (t s)"), pp[:],
                                biasg_sb[:, d, gc : gc + 1])
                        else:
                            nc.scalar.activation(
                                ptmp[:].rearrange("p t s -> p (t s)"), pp[:], ACTF.Identity,
                                bias=biasg_sb[:, d, gc : gc + 1], scale=1.0,
                            )
                        # pp rows r = nq*512 + (t', s): t = 8*nq + t'
                        nc.sync.dma_start(
                            P_dram[d, 8 * nq : 8 * nq + 8, :, gc, :].rearrange(
                                "t p s -> p t s"),
                            ptmp[:],
                        )

        # ================= stage D: recurrence (fp16 matmuls) ================
        for d in range(2):
            init_slot = 0 if d == 0 else B + 1
            for kc in range(KC):
                nc.sync.dma_start(hAll[:, d, kc, init_slot, :], h0_d[d, :, kc, :])
                nc.sync.dma_start(cT[:, d, kc, :], c0_d[d, :, kc, :])

        with tc.tile_pool(name="rec", bufs=1) as rec, \
             tc.tile_pool(name="psD", bufs=1, space="PSUM") as psum:
            for step in range(B):
                for d in range(2):
                    tl = step if d == 0 else B - 1 - step  # x/P time index
                    rd_slot = step if d == 0 else B + 1 - step
                    wr_slot = step + 1 if d == 0 else B - step
                    pslice = rec.tile([128, GC, 64], F32, tag=f"pslice{d}", bufs=3)
                    nc.sync.dma_start(pslice[:], P_dram[d, tl, :, :, :])
                    gp = psum.tile([128, 512], F32, tag=f"gp{d}", bufs=2)
                    for gc in range(GC):
                        for kc in range(KC):
                            nc.tensor.matmul(
                                gp[:, gc * 64 : (gc + 1) * 64],
                                lhsT=whh_sb[:, d, kc, gc * 128 : (gc + 1) * 128],
                                rhs=hAll[:, d, kc, rd_slot, :],
                                start=(kc == 0),
                                stop=(kc == KC - 1),
                            )
                    pre = rec.tile([128, GC, 64], F32, tag=f"pre{d}", bufs=3)
                    nc.vector.tensor_tensor(
                        pre[:],
                        gp[:].rearrange("p (g s) -> p g s", g=GC),
                        pslice[:],
                        op=OP.add,
                    )
                    gates = rec.tile([128, 512], F32, tag=f"gates{d}", bufs=3)
                    prefl = pre[:].rearrange("p g s -> p (g s)")
                    nc.scalar.activation(gates[:, 0:384], prefl[:, 0:384], ACTF.Sigmoid)
                    nc.scalar.activation(gates[:, 384:512], prefl[:, 384:512], ACTF.Tanh)
                    # gate layout (after host row-permute): i f o g, each 128 wide
                    ct = cT[:, d, :, :].rearrange("p kc s -> p (kc s)")
                    tmp = rec.tile([128, 128], F32, tag=f"tmp{d}", bufs=3)
                    nc.vector.tensor_tensor(tmp[:], gates[:, 0:128], gates[:, 384:512], op=OP.mult)
                    nc.gpsimd.tensor_tensor(ct, gates[:, 128:256], ct, op=OP.mult)
                    nc.gpsimd.tensor_tensor(ct, ct, tmp[:], op=OP.add)
                    tc2 = rec.tile([128, 128], F32, tag=f"tc{d}", bufs=3)
                    nc.scalar.activation(tc2[:], ct, ACTF.Tanh)
                    nc.vector.tensor_tensor(
                        hAll[:, d, :, wr_slot, :],
                        gates[:, 256:384].rearrange("p (kc s) -> p kc s", kc=KC),
                        tc2[:].rearrange("p (kc s) -> p kc s", kc=KC),
                        op=OP.mult,
                    )

        # ================= stage E-F: feats, transpose, AllGather =============
        fb_local = dram.tile([R, T], F32)
        with tc.tile_pool(name="fstage", bufs=1) as fs, \
             tc.tile_pool(name="psE", bufs=1, space="PSUM") as psum:
            feats_sb = fs.tile([T, R], F32)
            for nq in range(NQ):
                fp = psum.tile([T, 512], F32, tag="fpsum", bufs=2)
                for d in range(2):
                    for kc in range(KC):
                        q = d * 2 + kc
                        hflat = hAll[:, d, kc, 1 : B + 1, :].rearrange("p t s -> p (t s)")
                        nc.tensor.matmul(
                            fp[:],
                            lhsT=wout_sb[:, q, :],
                            rhs=hflat[:, nq * 512 : (nq + 1) * 512],
                            start=(q == 0),
                            stop=(q == 3),
                        )
                nc.scalar.activation(
                    feats_sb[:, nq * 512 : (nq + 1) * 512], fp[:], ACTF.Identity,
                    bias=bout_sb[:], scale=1.0,
                )

            ftr = fs.tile([128, 16, T], F32)
            for k in range(16):
                pt2 = psum.tile([128, T], F32, tag="ftpose", bufs=2)
                nc.tensor.transpose(
                    pt2[:], feats_sb[:, k * 128 : (k + 1) * 128], identity[:T, :T]
                )
                if k % 2 == 0:
                    nc.vector.tensor_copy(ftr[:, k, :], pt2[:])
                else:
                    nc.scalar.copy(ftr[:, k, :], pt2[:])
            nc.sync.dma_start(fb_local[:].rearrange("(k p) j -> p k j", p=128), ftr[:])

        # M[b, s, j, i] = trans[j, i] + feat[b, 64c + s, j] for this core's block
        Mown_dram = dram.tile([B, SB * T * T], F32)
        with tc.tile_pool(name="mstage", bufs=1) as ms:
            fown = ms.tile([B, SB, T], F32)
            nc.sync.dma_start(
                fown[:],
                fb_local[:].rearrange("(b s) j -> b s j", b=B),
            )
            Mown = ms.tile([B, SB, T, T], F32)
            nc.vector.tensor_tensor(
                Mown[:],
                trans_sb[:, None, :, :].to_broadcast([B, SB, T, T]),
                fown[:, :, :, None].to_broadcast([B, SB, T, T]),
                op=OP.add,
            )
            nc.sync.dma_start(
                Mown_dram[:], Mown[:].rearrange("b s j i -> b (s j i)")
            )
        M_full = dram.tile([NCORES, B, SB * T * T], F32, addr_space="Shared")
        nc.gpsimd.collective_compute(
            "AllGather",
            OP.bypass,
            replica_groups=[list(range(NCORES))],
            ins=[Mown_dram[:].opt()],
            outs=[M_full[:].opt()],
        )

        # ================= stage G: viterbi delta scan (replicated) ==========
        # 3-op scan, all on DVE:
        #   sc[b,j,i] = trans[j,i] + delta_{s-1}[b,i];  red = max_i sc;
        #   delta_s   = red + feat_s
        nc.sync.dma_start(dwin[:, 0:T], delta0_d[:])
        with tc.tile_pool(name="vit", bufs=1) as vit:
            for ch in range(S // VCH):
                Mch = vit.tile([B, VCH, T, T], F32, tag="Mch", bufs=3)
                nc.sync.dma_start(
                    Mch[:].rearrange("b s j i -> b (s j i)"), M_full[ch, :, :]
                )
                for sl in range(VCH):
                    s = ch * VCH + sl
                    if s == 0:
                        continue
                    sc = vit.tile([B, T, T], F32, tag="sc", bufs=6)
                    nc.vector.tensor_tensor(
                        sc[:],
                        Mch[:, sl, :, :],
                        dwin[:, (s - 1) * T : s * T][:, None, :].to_broadcast([B, T, T]),
                        op=OP.add,
                    )
                    nc.vector.tensor_reduce(
                        dwin[:, s * T : (s + 1) * T], sc[:], axis=AX.X, op=OP.max
                    )

        nc.sync.dma_start(deltas_d[:], dwin[:])

    nc.finalize()
    return nc


def _pack_inputs(inputs):
    f32 = np.float32
    sentence = np.asarray(inputs["sentence"]).astype(np.int32)
    embed = np.ascontiguousarray(np.asarray(inputs["embed"], f32))
    h0 = np.asarray(inputs["h0"], f32)
    c0 = np.asarray(inputs["c0"], f32)
    w_out = np.asarray(inputs["w_out"], f32)
    b_out = np.asarray(inputs["b_out"], f32)
    transitions = np.asarray(inputs["transitions"], f32)

    # gate row permutation: torch (i, f, g, o) -> device (i, f, o, g)
    perm = np.concatenate([
        np.arange(0, 256), np.arange(256, 512),
        np.arange(768, 1024), np.arange(512, 768),
    ])

    def pack_dir(w_ih, w_hh, b_ih, b_hh):
        w_ih = np.asarray(w_ih, f32)[perm]
        w_hh = np.asarray(w_hh, f32)[perm]
        bias = (np.asarray(b_ih, f32) + np.asarray(b_hh, f32))[perm]
        return (
            np.ascontiguousarray(w_ih.T),                       # (E, G)
            np.ascontiguousarray(w_hh.T),                       # (H, G)
            np.ascontiguousarray(bias.reshape(GC, 128).T),      # (128, GC)
        )

    wihT_f, whhT_f, biasg_f = pack_dir(inputs["w_ih_f"], inputs["w_hh_f"],
                                       inputs["b_ih_f"], inputs["b_hh_f"])
    wihT_b, whhT_b, biasg_b = pack_dir(inputs["w_ih_b"], inputs["w_hh_b"],
                                       inputs["b_ih_b"], inputs["b_hh_b"])
    wihT = np.ascontiguousarray(np.stack([wihT_f, wihT_b])).astype(np.float16)
    whhT = np.ascontiguousarray(np.stack([whhT_f, whhT_b])).astype(np.float16)
    biasg = np.ascontiguousarray(np.stack([biasg_f, biasg_b]))

    # w_out (T, HD): lhsT chunks [128, q, T], q = d*2+kc over the hf|hb concat
    woutT = np.ascontiguousarray(
        w_out.T.reshape(4, 128, T).transpose(1, 0, 2)).astype(np.float16)
    boutc = np.ascontiguousarray(b_out.reshape(T, 1))
    transtile = np.ascontiguousarray(np.broadcast_to(transitions[None], (B, T, T)))
    delta0 = np.full((B, T), NEG, f32)
    delta0[:, START] = 0.0

    shared = dict(embed=embed, wihT=wihT, whhT=whhT, biasg=biasg,
                  woutT=woutT, bout=boutc, transtile=transtile, delta0=delta0)

    in_maps = []
    for c in range(NCORES):
        sl = slice(c * SB, (c + 1) * SB)
        sent = np.ascontiguousarray(sentence[:, sl].reshape(-1))  # (R,) t-major

        def pack_state(st):
            sh = np.asarray(st, f32)[:, sl, :]                      # (2, SB, H)
            sh = sh.transpose(0, 2, 1).reshape(2, KC, 128, SB)      # (2, kc, p, s)
            return np.ascontiguousarray(sh.transpose(0, 2, 1, 3))   # (2, p, kc, s)

        m = dict(shared)
        m["sent"] = sent
        m["h0t"] = pack_state(h0).astype(np.float16)
        m["c0t"] = pack_state(c0)
        in_maps.append(m)
    return in_maps, transitions


def _run(inputs, trace=False):
    if "nc" not in _CACHE:
        _CACHE["nc"] = _build()
    nc = _CACHE["nc"]
    in_maps, transitions = _pack_inputs(inputs)
    res = run_bass_kernel_spmd(nc, in_maps, core_ids=list(range(NCORES)), trace=trace)
    return res, transitions


def kernel(**inputs):
    res, transitions = _run(inputs, trace=False)
    deltas = res.results[0]["deltas"].reshape(B, S, T)
    return _finish(deltas, transitions)


def _finish(deltas, transitions):
    score = deltas[:, S - 1].max(-1).astype(np.float32)
    nxt = deltas[:, S - 1].argmax(-1)
    path = np.empty((B, S), np.int64)
    path[:, S - 1] = nxt
    tr = np.asarray(transitions, np.float32)
    for s in range(S - 1, 0, -1):
        scb = tr[nxt] + deltas[:, s - 1]
        nxt = scb.argmax(-1)
        path[:, s - 1] = nxt
    return score, path.astype(np.int32)
